# revision 1
# baseline (speedup 1.0000x reference)
"""BertSelfAttention (relative_key_query position embeddings) on 8 TRN2 NeuronCores.

Full inputs in, full output out.  Sharding: data-parallel over batch (4) x
tensor-parallel over head-groups (2 groups of 6 heads) = 8 cores, SPMD (one
NEFF, per-core input slices).

Math (per batch b, head h):
  q = hs @ Wq + bq ; k, v likewise            [S, 64] per head
  scores[l,r] = q[l]@k[r] + q[l]@D[l-r+M-1] + k[r]@D[l-r+M-1]
  probs = softmax(scores/8 + mask) ; ctx = probs @ v

Device algorithm (transposed orientation S[r,l], softmax over partitions):
  * The relative-position terms are handled with "band tables":
      Atab_b[p, c] = q[128b+p] . Drev_pad[896-128b+c]   (c in [0,1152))
    A row-pitch-1151 (instead of 1152) strided read of Atab yields
      qpos_b[p, r] = q[l] . D[l-r+1023]  exactly (regular DMA, on-chip,
    batched: one SBUF->SBUF DMA per table per head).
    Same construction with D (unreversed) and k gives kposT directly in
    [r, l] orientation; qpos tiles are transpose-accumulated into the score
    PSUM via identity matmuls on the tensor engine.
  * exp((S)*0.125 + mask) fused on ACT (mask enters as per-partition bias).
  * No row-max subtraction: |scores/8| <~ 2 for this distribution, exp is
    safely in fp32 range; softmax is algebraically identical.
  * PV uses lhsT = [v | 1]: row 64 of the output accumulates the softmax
    denominator for free; division happens after the final transpose.
  * bv is folded in on the host (ctx = ctx_nobv + bv since rows of probs
    sum to 1); bq/bk are applied on-device as per-partition biases.
"""

import numpy as np
import ml_dtypes

import concourse.bass as bass
import concourse.mybir as mybir
import concourse.tile as tile
from concourse import bacc
from concourse.bass_utils import run_bass_kernel_spmd
from concourse.masks import make_identity

F32 = mybir.dt.float32
BF16 = mybir.dt.bfloat16
AF = mybir.ActivationFunctionType

B, S, H = 4, 1024, 768
NH, HD = 12, 64
MAXP = 1024
NCORES = 8
HPC = 6           # heads per core
DW = HPC * HD     # 384 out-dims per core
P = 128
NB = S // P       # 8 blocks of 128 along l and r
BAND = 1152       # band width per block (1151 needed, padded to 1152)
JW = 2048         # padded dist table width

_CACHE: dict = {}

# experiment toggles (affect _build; cache-keyed).  Defaults = best found
# via TimelineSim cost-model A/B (375us -> 245us single-shot predicted).
OPTS = {
    "pv_interleave": False,   # emit PV matmuls inside the scores loop
    "kpos_pe": True,          # add kposT via identity-matmul instead of DVE
    "act_evac": (2, 1),       # table evac on ACT when idx % m < k (else DVE)
    "evac_split": False,      # split each table evac across ACT+DVE halves
    "ctx_dve": True,          # ctxT copy on DVE instead of ACT
    "fp8_tab": True,          # position tables/shift tiles in fp8e4m3
    "tab_bufs": 2,
    "lookahead": 1,           # emit tables(h+1) before scores(h)
    "proj_order": True,       # head-major q/k projection order + split loads
    "fin_inline": False,      # emit output transposes inside the head loop
    "psum_split": False,      # scores get their own 2-bank psum tag;
                              # tables 1x3-bank, PV accumulator halved
    "limit": "full",          # debug: "proj" | "tables" | "scores" | "full"
}


def _diag_ap3(ap: bass.AP, nblk: int, pitch: int, part_n: int, free_n: int,
              off: int) -> bass.AP:
    """Batched diagonal view of a [part_n, nblk, pitch] tile:
    out[p, b, f] = X[p, b, f - p + off]."""
    d = ap.copy()
    v = d.ap
    while len(v) > 0:
        v.pop()
    v.append([nblk * pitch - 1, part_n])
    v.append([pitch, nblk])
    v.append([1, free_n])
    d.offset = ap.offset + off
    return d


def _build(reps: int = 1):
    key = ("nc", reps, tuple(sorted(OPTS.items())))
    if key in _CACHE:
        return _CACHE[key]

    nc = bacc.Bacc("TRN2", target_bir_lowering=False, debug=False)

    hst_d = nc.dram_tensor("hst", [P, 6, S], BF16, kind="ExternalInput")
    wq_d = nc.dram_tensor("wq", [P, 6, DW], BF16, kind="ExternalInput")
    wk_d = nc.dram_tensor("wk", [P, 6, DW], BF16, kind="ExternalInput")
    wv_d = nc.dram_tensor("wv", [P, 6, DW], BF16, kind="ExternalInput")
    drev_d = nc.dram_tensor("drevt", [P, JW], BF16, kind="ExternalInput")
    dt_d = nc.dram_tensor("dtt", [P, JW], BF16, kind="ExternalInput")
    mask_d = nc.dram_tensor("maskc", [P, NB], F32, kind="ExternalInput")
    bqk_d = nc.dram_tensor("bqkc", [P, 6], F32, kind="ExternalInput")
    out_d = nc.dram_tensor("out", [P, NB, DW], F32, kind="ExternalOutput")

    with tile.TileContext(nc) as tc:
        with tc.tile_pool(name="persist", bufs=1) as pp:
            drevt = pp.tile([P, JW], BF16)
            dtt = pp.tile([P, JW], BF16)
            maskt = pp.tile([P, NB], F32)
            bqkt = pp.tile([P, 6], F32)
            identb = pp.tile([P, P], BF16)
            ident8 = pp.tile([P, P], mybir.dt.float8e4)
            identf = pp.tile([P, P], F32)
            qT = pp.tile([P, 3, S], BF16)     # head h: [64*(h%2):.., h//2, :]
            kT = pp.tile([P, 3, S], BF16)
            vsb = pp.tile([P, NB, HPC, HD + 1], BF16)  # [r%128, r//128, h, d|1]
            ctxT = pp.tile([HD + 1, HPC, S], F32)
            outsb = pp.tile([P, NB, DW], F32)

            nc.sync.dma_start(drevt[:], drev_d[:])
            nc.sync.dma_start(dtt[:], dt_d[:])
            nc.sync.dma_start(maskt[:], mask_d[:])
            nc.sync.dma_start(bqkt[:], bqk_d[:])
            make_identity(nc, identb[:])
            make_identity(nc, ident8[:])
            make_identity(nc, identf[:])
            nc.vector.memset(vsb[:, :, :, HD], 1.0)

            for rep in range(reps):
                _emit_iteration(nc, tc, rep,
                                hst_d, wq_d, wk_d, wv_d, out_d,
                                drevt, dtt, maskt, bqkt,
                                identb, ident8, identf, qT, kT, vsb, ctxT,
                                outsb)

    nc.compile()
    _CACHE[key] = nc
    return nc


def _emit_iteration(nc, tc, rep, hst_d, wq_d, wk_d, wv_d, out_d,
                    drevt, dtt, maskt, bqkt,
                    identb, ident8, identf, qT, kT, vsb, ctxT, outsb):
    TAB = mybir.dt.float8e4 if OPTS["fp8_tab"] else BF16
    identt = ident8 if OPTS["fp8_tab"] else identb
    tab_bufs = OPTS.get("tab_bufs", 2) if OPTS["fp8_tab"] else 1
    # ---------------- Phase 1: projections ----------------
    with (
        tc.tile_pool(name=f"proj{rep}", bufs=1) as prp,
        tc.tile_pool(name=f"projps{rep}", bufs=OPTS.get("proj_bufs", 2), space="PSUM") as prps,
    ):
        hst = prp.tile([P, 6, S], BF16)
        wqt = prp.tile([P, 6, DW], BF16)
        wkt = prp.tile([P, 6, DW], BF16)
        wvt = prp.tile([P, 6, DW], BF16)
        if OPTS.get("proj_order", False):
            # split loads so the first matmuls start early
            for kc in range(6):
                nc.sync.dma_start(hst[:, kc, :], hst_d[:, kc, :])
                nc.sync.dma_start(wqt[:, kc, :], wq_d[:, kc, :])
                nc.sync.dma_start(wkt[:, kc, :], wk_d[:, kc, :])
            nc.sync.dma_start(wvt[:], wv_d[:])
            # head-major: q then k per m-tile, so tables(0)/(1) unblock first
            order = [(pi, m) for m in range(3) for pi in range(2)]
        else:
            nc.sync.dma_start(hst[:], hst_d[:])
            nc.sync.dma_start(wqt[:], wq_d[:])
            nc.sync.dma_start(wkt[:], wk_d[:])
            nc.sync.dma_start(wvt[:], wv_d[:])
            order = [(pi, m) for pi in range(2) for m in range(3)]

        # qT / kT: out[M=128 outdims (2 heads), N=512 tokens]
        for pi, m in order:
            wt, dst = ((wqt, qT), (wkt, kT))[pi]
            for nh2 in range(2):
                ps = prps.tile([P, 512], F32, tag="pqk")
                for kc in range(6):
                    nc.tensor.matmul(
                        ps[:],
                        wt[:, kc, m * P:(m + 1) * P],
                        hst[:, kc, nh2 * 512:(nh2 + 1) * 512],
                        start=(kc == 0), stop=(kc == 5),
                    )
                nc.scalar.activation(
                    dst[:, m, nh2 * 512:(nh2 + 1) * 512], ps[:],
                    AF.Identity, bias=bqkt[:, 3 * pi + m:3 * pi + m + 1],
                    scale=1.0,
                )
        # v: out[M=128 tokens, N=384 outdims]
        for t in range(NB):
            ps = prps.tile([P, DW], F32, tag="pv")
            for kc in range(6):
                nc.tensor.matmul(
                    ps[:],
                    hst[:, kc, t * P:(t + 1) * P],
                    wvt[:, kc, :],
                    start=(kc == 0), stop=(kc == 5),
                )
            nc.vector.tensor_copy(
                vsb[:, t, :, 0:HD],
                ps[:].rearrange("p (h d) -> p h d", h=HPC),
            )

    if OPTS["limit"] == "proj":
        return
    # ---------------- Phase 2: per-head attention ----------------
    shift_bufs = 1 + max(1, int(OPTS["lookahead"])) if OPTS["fp8_tab"] else 2
    with (
        tc.tile_pool(name=f"wtab{rep}", bufs=tab_bufs) as wtab,
        tc.tile_pool(name=f"wqpos{rep}", bufs=shift_bufs) as wqpos,
        tc.tile_pool(name=f"wkpt{rep}",
                     bufs=shift_bufs if OPTS["fp8_tab"] else 1) as wkpt,
        tc.tile_pool(name=f"wexp{rep}", bufs=OPTS.get("exp_bufs", 10)) as wexp,
        tc.tile_pool(name=f"hps{rep}", bufs=2, space="PSUM") as hps,
        tc.tile_pool(name=f"htab{rep}", bufs=1, space="PSUM") as htab,
        tc.tile_pool(name=f"cps{rep}", bufs=1, space="PSUM") as cps,
    ):
        split = OPTS["psum_split"]
        def emit_tables(h):
            base = (h % 2) * 64
            tl = h // 2
            qTh = qT[base:base + 64, tl, :]
            kTh = kT[base:base + 64, tl, :]
            # --- band tables + batched on-chip diagonal shifts
            atab = wtab.tile([P, NB, BAND], TAB, tag="atab")
            btab = wtab.tile([P, NB, BAND], TAB, tag="btab")
            for blk in range(NB):
                j0 = 896 - P * blk
                for side in range(2):
                    ps = (htab if split else hps).tile(
                        [P, BAND], F32, tag="work")
                    lhsT = (qTh if side == 0 else kTh)[:, blk * P:(blk + 1) * P]
                    rhs_t = (drevt if side == 0 else dtt)[base:base + 64, :]
                    for c0, c1 in ((0, 512), (512, 1024), (1024, BAND)):
                        nc.tensor.matmul(
                            ps[:, c0:c1], lhsT,
                            rhs_t[:, j0 + c0:j0 + c1],
                            start=True, stop=True,
                            skip_group_check=True,
                        )
                    tabs = (atab if side == 0 else btab)[:, blk, :]
                    if OPTS["evac_split"]:
                        nc.scalar.copy(tabs[:, 0:576], ps[:, 0:576])
                        nc.vector.tensor_copy(tabs[:, 576:BAND], ps[:, 576:BAND])
                    else:
                        em, ek = OPTS["act_evac"]
                        if (blk * 2 + side) % em < ek:
                            nc.scalar.copy(tabs, ps[:])
                        else:
                            nc.vector.tensor_copy(tabs, ps[:])
            if OPTS["limit"] == "tabnoshift":
                nc.vector.tensor_copy(ctxT[0:P // 2, h, 0:S], atab[0:P // 2, 0, 0:S])
                nc.vector.tensor_copy(ctxT[0:1, h, 0:S], btab[0:1, 0, 0:S])
                return None
            # shift DMAs per side per head (optionally split by r-half so
            # the first score blocks unblock after half the transfer)
            qpos = wqpos.tile([P, NB, S], TAB, tag="qpos")
            kpt = wkpt.tile([P, NB, S], TAB, tag="kpt")
            nsp = int(OPTS.get("shift_split", 1))
            hw_ = S // nsp
            for sp in range(nsp):
                nc.sync.dma_start(
                    qpos[:, :, sp * hw_:(sp + 1) * hw_],
                    _diag_ap3(atab[:], NB, BAND, P, hw_, P - 1 + sp * hw_))
                nc.sync.dma_start(
                    kpt[:, :, sp * hw_:(sp + 1) * hw_],
                    _diag_ap3(btab[:], NB, BAND, P, hw_, P - 1 + sp * hw_))
            if OPTS["limit"] == "tables":
                nc.vector.tensor_copy(ctxT[0:P // 2, h, 0:S], qpos[0:P // 2, 0, :])
                nc.vector.tensor_copy(ctxT[0:1, h, 0:S], kpt[0:1, 0, :])
                return None
            return qpos, kpt

        def emit_scores(h, qpos, kpt):
            base = (h % 2) * 64
            tl = h // 2
            qTh = qT[base:base + 64, tl, :]
            kTh = kT[base:base + 64, tl, :]
            # --- scores, softmax numerator (+PV accumulate)
            cac = None
            if not split:
                cac = cps.tile([HD + 1, S], F32, tag="ctxacc")
            expts = []
            for j in range(NB):
                if split:
                    sps = hps.tile([P, S], F32, tag="sc")
                    sS = sps[:]
                else:
                    sps = hps.tile([P, BAND], F32, tag="work")
                    sS = sps[:, 0:S]
                for nh2 in range(2):
                    nc.tensor.matmul(
                        sS[:, nh2 * 512:(nh2 + 1) * 512],
                        kTh[:, j * P:(j + 1) * P],
                        qTh[:, nh2 * 512:(nh2 + 1) * 512],
                        start=True, stop=False,
                        skip_group_check=True,
                    )
                kmode = OPTS["kpos_pe"]
                use_pe = kmode is True or (kmode == "hybrid" and j % 2 == 0)
                last_stop = not use_pe
                for blk in range(NB):
                    nc.tensor.matmul(
                        sS[:, blk * P:(blk + 1) * P],
                        qpos[:, blk, j * P:(j + 1) * P],
                        identt[:],
                        start=False, stop=last_stop,
                        skip_group_check=True,
                    )
                if use_pe:
                    for nh2 in range(2):
                        nc.tensor.matmul(
                            sS[:, nh2 * 512:(nh2 + 1) * 512],
                            identt[:],
                            kpt[:, j, nh2 * 512:(nh2 + 1) * 512],
                            start=False, stop=True,
                            skip_group_check=True,
                        )
                else:
                    nc.vector.tensor_tensor(
                        sS, sS, kpt[:, j, :], mybir.AluOpType.add)
                expt = wexp.tile([P, S], BF16, tag="expt")
                nc.scalar.activation(
                    expt[:], sS, AF.Exp,
                    bias=maskt[:, j:j + 1], scale=0.125)
                expts.append(expt)
                if OPTS["pv_interleave"]:
                    for nh2 in range(2):
                        nc.tensor.matmul(
                            cac[:, nh2 * 512:(nh2 + 1) * 512],
                            vsb[:, j, h, :],
                            expt[:, nh2 * 512:(nh2 + 1) * 512],
                            start=(j == 0), stop=(j == NB - 1),
                            skip_group_check=True,
                        )

            # --- PV (+denominator via ones column)
            if OPTS["limit"] == "scores":
                return
            if split:
                for nh2 in range(2):
                    cach = cps.tile([HD + 1, 512], F32, tag="ctxacc")
                    for j in range(NB):
                        nc.tensor.matmul(
                            cach[:],
                            vsb[:, j, h, :],
                            expts[j][:, nh2 * 512:(nh2 + 1) * 512],
                            start=(j == 0), stop=(j == NB - 1),
                            skip_group_check=True,
                        )
                    cp = nc.vector.tensor_copy if OPTS["ctx_dve"] else nc.scalar.copy
                    cp(ctxT[:, h, nh2 * 512:(nh2 + 1) * 512], cach[:])
            else:
                if not OPTS["pv_interleave"]:
                    for j in range(NB):
                        for nh2 in range(2):
                            nc.tensor.matmul(
                                cac[:, nh2 * 512:(nh2 + 1) * 512],
                                vsb[:, j, h, :],
                                expts[j][:, nh2 * 512:(nh2 + 1) * 512],
                                start=(j == 0), stop=(j == NB - 1),
                                skip_group_check=True,
                            )
                if OPTS["ctx_dve"]:
                    nc.vector.tensor_copy(ctxT[:, h, :], cac[:])
                else:
                    nc.scalar.copy(ctxT[:, h, :], cac[:])
            if OPTS["fin_inline"]:
                # output transposes ride in borrowed "work" PSUM slots and
                # overlap the next head's tables
                for lt in range(NB):
                    ctp = hps.tile([P, BAND], F32, tag="work")
                    ct = ctp[:, 0:HD + 1]
                    nc.tensor.matmul(
                        ct,
                        ctxT[:, h, lt * P:(lt + 1) * P],
                        identf[0:HD + 1, 0:HD + 1],
                        start=True, stop=True,
                        skip_group_check=True,
                    )
                    rc = wexp.tile([P, 1], F32, tag="rc")
                    nc.vector.reciprocal(rc[:], ct[:, HD:HD + 1])
                    nc.vector.tensor_scalar_mul(
                        outsb[:, lt, h * HD:(h + 1) * HD],
                        ct[:, 0:HD], rc[:])

        depth = int(OPTS["lookahead"])
        if depth > 0:
            pend = {}
            for h in range(min(depth, HPC)):
                pend[h] = emit_tables(h)
            for h in range(HPC):
                if h + depth < HPC:
                    pend[h + depth] = emit_tables(h + depth)
                sh = pend.pop(h)
                if sh is not None:
                    emit_scores(h, *sh)
        else:
            for h in range(HPC):
                sh = emit_tables(h)
                if sh is not None:
                    emit_scores(h, *sh)

        if OPTS["fin_inline"] and OPTS["limit"] == "full":
            nc.sync.dma_start(out_d[:], outsb[:])

    if OPTS["limit"] != "full" or OPTS["fin_inline"]:
        return
    # ---------------- Phase 3: output assembly ----------------
    with (
        tc.tile_pool(name=f"fin{rep}", bufs=4) as fin,
        tc.tile_pool(name=f"fps{rep}", bufs=4, space="PSUM") as fps,
    ):
        for h in range(HPC):
            for lt in range(NB):
                ct = fps.tile([P, HD + 1], F32, tag="ctps")
                nc.tensor.matmul(
                    ct[:],
                    ctxT[:, h, lt * P:(lt + 1) * P],
                    identf[0:HD + 1, 0:HD + 1],
                    start=True, stop=True,
                    skip_group_check=True,
                )
                rc = fin.tile([P, 1], F32, tag="rc")
                nc.vector.reciprocal(rc[:], ct[:, HD:HD + 1])
                nc.vector.tensor_scalar_mul(
                    outsb[:, lt, h * HD:(h + 1) * HD],
                    ct[:, 0:HD], rc[:])
        nc.sync.dma_start(out_d[:], outsb[:])


def build_in_maps(inputs):
    hs = np.asarray(inputs["hidden_states"], np.float32)
    am = np.asarray(inputs["attention_mask"], np.float32)
    Wq = np.asarray(inputs["Wq"], np.float32)
    Wk = np.asarray(inputs["Wk"], np.float32)
    Wv = np.asarray(inputs["Wv"], np.float32)
    bq = np.asarray(inputs["bq"], np.float32)
    bk = np.asarray(inputs["bk"], np.float32)
    de = np.asarray(inputs["dist_emb"], np.float32)

    bf = ml_dtypes.bfloat16

    # dist tables, padded to 2048 cols, duplicated on both partition halves
    drevt = np.zeros((64, JW), np.float32)
    drevt[:, :2047] = de[::-1].T
    dtt = np.zeros((64, JW), np.float32)
    dtt[:, :2047] = de.T
    drevt = np.concatenate([drevt, drevt], 0).astype(bf)
    dtt = np.concatenate([dtt, dtt], 0).astype(bf)

    in_maps = []
    for core in range(NCORES):
        b, g = divmod(core, 2)
        cols = slice(g * DW, (g + 1) * DW)
        hst = np.ascontiguousarray(hs[b].T).reshape(6, P, S)
        hst = np.ascontiguousarray(hst.transpose(1, 0, 2)).astype(bf)
        wqc = np.ascontiguousarray(
            Wq[:, cols].reshape(6, P, DW).transpose(1, 0, 2)).astype(bf)
        wkc = np.ascontiguousarray(
            Wk[:, cols].reshape(6, P, DW).transpose(1, 0, 2)).astype(bf)
        wvc = np.ascontiguousarray(
            Wv[:, cols].reshape(6, P, DW).transpose(1, 0, 2)).astype(bf)
        maskc = np.ascontiguousarray(am[b, 0, 0, :].reshape(NB, P).T)
        # col 0..2: bq m-tiles (128 dims each); col 3..5: bk m-tiles
        bqkc = np.concatenate(
            [bq[cols].reshape(3, P).T, bk[cols].reshape(3, P).T], axis=1)
        in_maps.append({
            "hst": hst, "wq": wqc, "wk": wkc, "wv": wvc,
            "drevt": drevt, "dtt": dtt,
            "maskc": maskc.astype(np.float32),
            "bqkc": np.ascontiguousarray(bqkc).astype(np.float32),
        })
    return in_maps


def kernel(hidden_states, attention_mask, Wq, bq, Wk, bk, Wv, bv, dist_emb):
    in_maps = build_in_maps({
        "hidden_states": hidden_states, "attention_mask": attention_mask,
        "Wq": Wq, "Wk": Wk, "Wv": Wv, "bq": bq, "bk": bk,
        "dist_emb": dist_emb,
    })
    bv = np.asarray(bv, np.float32)

    nc = _build()
    try:
        res = run_bass_kernel_spmd(nc, in_maps, core_ids=list(range(NCORES)))
    except Exception:
        # one retry for transient runtime/device hiccups
        res = run_bass_kernel_spmd(nc, in_maps, core_ids=list(range(NCORES)))

    out = np.empty((B, S, H), np.float32)
    for core in range(NCORES):
        b, g = divmod(core, 2)
        o = res.results[core]["out"]          # [128, 8, 384]
        out[b, :, g * DW:(g + 1) * DW] = o.transpose(1, 0, 2).reshape(S, DW)
    out += bv[None, None, :]
    return out



# revision 21
# speedup vs baseline: 20.4657x; 20.4657x over previous
"""BertSelfAttention (relative_key_query position embeddings) on 8 TRN2 NeuronCores.

Full inputs in, full output out.  Sharding: data-parallel over batch (4) x
tensor-parallel over head-groups (2 groups of 6 heads) = 8 cores, SPMD (one
NEFF, per-core input slices).

Math (per batch b, head h):
  q = hs @ Wq + bq ; k, v likewise            [S, 64] per head
  scores[l,r] = q[l]@k[r] + q[l]@D[l-r+M-1] + k[r]@D[l-r+M-1]
  probs = softmax(scores/8 + mask) ; ctx = probs @ v

Device algorithm (transposed orientation S[r,l], softmax over partitions):
  * The relative-position terms are handled with "band tables":
      Atab_b[p, c] = q[128b+p] . Drev_pad[896-128b+c]   (c in [0,1152))
    A row-pitch-1151 (instead of 1152) strided read of Atab yields
      qpos_b[p, r] = q[l] . D[l-r+1023]  exactly (regular DMA, on-chip,
    batched: one SBUF->SBUF DMA per table per head).
    Same construction with D (unreversed) and k gives kposT directly in
    [r, l] orientation; qpos tiles are transpose-accumulated into the score
    PSUM via identity matmuls on the tensor engine.
  * exp((S)*0.125 + mask) fused on ACT (mask enters as per-partition bias).
  * No row-max subtraction: |scores/8| <~ 2 for this distribution, exp is
    safely in fp32 range; softmax is algebraically identical.
  * PV uses lhsT = [v | 1]: row 64 of the output accumulates the softmax
    denominator for free; division happens after the final transpose.
  * bv is folded in on the host (ctx = ctx_nobv + bv since rows of probs
    sum to 1); bq/bk are applied on-device as per-partition biases.
"""

import numpy as np
import ml_dtypes

import concourse.bass as bass
import concourse.mybir as mybir
import concourse.tile as tile
from concourse import bacc
from concourse.bass_utils import run_bass_kernel_spmd
from concourse.masks import make_identity

F32 = mybir.dt.float32
BF16 = mybir.dt.bfloat16
AF = mybir.ActivationFunctionType

B, S, H = 4, 1024, 768
NH, HD = 12, 64
MAXP = 1024
NCORES = 8
HPC = 6           # heads per core
DW = HPC * HD     # 384 out-dims per core
P = 128
NB = S // P       # 8 blocks of 128 along l and r
BAND = 1152       # band width per block (1151 needed, padded to 1152)
JW = 2048         # padded dist table width

_CACHE: dict = {}

# experiment toggles (affect _build; cache-keyed).  Defaults = best found
# via TimelineSim cost-model A/B (375us -> 245us single-shot predicted).
OPTS = {
    "pv_interleave": False,   # emit PV matmuls inside the scores loop
    "kpos_pe": True,          # add kposT via identity-matmul instead of DVE
    "act_evac": (2, 1),       # table evac on ACT when idx % m < k (else DVE)
    "evac_split": False,      # split each table evac across ACT+DVE halves
    # per-(blk,side,chunk) evac engine rotation (act/dve; gp can't see PSUM)
    "evac_pat": ("dve", "act"),
    "work_bufs": 2,
    "tab_chunk": 1152,        # 1152 = one full-band evac per (blk, side)
    "tab_dr": False,          # fp8 DoubleRow tables (no real win at K=64)
    "proj_dr": False,         # fp8 q/k projections fail the 2e-2 gate
    "ctx_dve": True,          # ctxT copy on DVE instead of ACT
    "ctx_gp": False,           # ctxT copy on Pool/GPSIMD (overrides ctx_dve)
    "fin_bf16": True,         # ctxT + output transposes in bf16 (1cyc PE)
    "fp8_tab": True,          # position tables/shift tiles in fp8e4m3
    "tab_bufs": 2,
    "lookahead": 1,           # emit tables(h+1) before scores(h)
    "proj_order": True,       # head-major q/k projection order + split loads
    "fin_inline": False,      # emit output transposes inside the head loop
    "psum_split": False,      # scores get their own 2-bank psum tag;
                              # tables 1x3-bank, PV accumulator halved
    "limit": "full",          # debug: "proj" | "tables" | "scores" | "full"
}


def _diag_ap3(ap: bass.AP, nblk: int, pitch: int, part_n: int, free_n: int,
              off: int) -> bass.AP:
    """Batched diagonal view of a [part_n, nblk, pitch] tile:
    out[p, b, f] = X[p, b, f - p + off]."""
    d = ap.copy()
    v = d.ap
    while len(v) > 0:
        v.pop()
    v.append([nblk * pitch - 1, part_n])
    v.append([pitch, nblk])
    v.append([1, free_n])
    d.offset = ap.offset + off
    return d


def _flat_ap(ap: bass.AP, dims, off: int) -> bass.AP:
    """AP with explicit flattened dims/offset (elements)."""
    d = ap.copy()
    v = d.ap
    while len(v) > 0:
        v.pop()
    for st, n in dims:
        v.append([st, n])
    d.offset = ap.offset + off
    return d


def _build(reps: int = 1):
    key = ("nc", reps, repr(sorted(OPTS.items(), key=lambda kv: kv[0])))
    if key in _CACHE:
        return _CACHE[key]

    nc = bacc.Bacc("TRN2", target_bir_lowering=False, debug=False)

    hst_d = nc.dram_tensor("hst", [P, 6, S], BF16, kind="ExternalInput")
    wq_d = nc.dram_tensor("wq", [P, 6, DW], BF16, kind="ExternalInput")
    wk_d = nc.dram_tensor("wk", [P, 6, DW], BF16, kind="ExternalInput")
    wv_d = nc.dram_tensor("wv", [P, 6, DW], BF16, kind="ExternalInput")
    drev_d = nc.dram_tensor("drevt", [P, JW], BF16, kind="ExternalInput")
    dt_d = nc.dram_tensor("dtt", [P, JW], BF16, kind="ExternalInput")
    hs8_d = nc.dram_tensor("hs8", [P, 6, S], mybir.dt.float8e4,
                           kind="ExternalInput")
    wq8_d = nc.dram_tensor("wq8", [P, 6, DW], mybir.dt.float8e4,
                           kind="ExternalInput")
    wk8_d = nc.dram_tensor("wk8", [P, 6, DW], mybir.dt.float8e4,
                           kind="ExternalInput")
    dr8_d = nc.dram_tensor("dr8", [P, 2, JW], mybir.dt.float8e4,
                           kind="ExternalInput")
    dt8_d = nc.dram_tensor("dt8", [P, 2, JW], mybir.dt.float8e4,
                           kind="ExternalInput")
    mask_d = nc.dram_tensor("maskc", [P, NB], F32, kind="ExternalInput")
    bqk_d = nc.dram_tensor("bqkc", [P, 6], F32, kind="ExternalInput")
    out_d = nc.dram_tensor("out", [P, NB, DW], F32, kind="ExternalOutput")

    with tile.TileContext(nc) as tc:
        with tc.tile_pool(name="persist", bufs=1) as pp:
            drevt = pp.tile([P, JW], BF16)
            dtt = pp.tile([P, JW], BF16)
            maskt = pp.tile([P, NB], F32)
            bqkt = pp.tile([P, 6], F32)
            identb = pp.tile([P, P], BF16)
            ident8 = pp.tile([P, P], mybir.dt.float8e4)
            identf = pp.tile([P, P], F32)
            qT = pp.tile([P, 3, S], BF16)     # head h: [64*(h%2):.., h//2, :]
            kT = pp.tile([P, 3, S], BF16)
            vsb = pp.tile([P, NB, HPC, HD + 1], BF16)  # [r%128, r//128, h, d|1]
            dr8s = pp.tile([P, 2, JW], mybir.dt.float8e4)
            dt8s = pp.tile([P, 2, JW], mybir.dt.float8e4)
            qT8f = pp.tile([P, 3, S], mybir.dt.float8e4)
            kT8f = pp.tile([P, 3, S], mybir.dt.float8e4)
            qT8 = pp.tile([P, 2, 2, S], mybir.dt.float8e4)
            kT8 = pp.tile([P, 2, 2, S], mybir.dt.float8e4)
            ctxT = pp.tile([HD + 1, HPC, S],
                           BF16 if OPTS["fin_bf16"] else F32)
            outsb = pp.tile([P, NB, DW], F32)

            if not OPTS["tab_dr"]:
                nc.sync.dma_start(drevt[:], drev_d[:])
                nc.sync.dma_start(dtt[:], dt_d[:])
            nc.sync.dma_start(maskt[:], mask_d[:])
            nc.sync.dma_start(bqkt[:], bqk_d[:])
            make_identity(nc, identb[:])
            make_identity(nc, identf[:])
            if OPTS["tab_dr"]:
                # tables are built from 16x-scaled D; fold 1/16 into the
                # fp8 identity used by the qpos/kpos injects
                nc.gpsimd.memset(ident8[:], 0.0)
                nc.gpsimd.affine_select(
                    out=ident8[:], in_=ident8[:],
                    compare_op=mybir.AluOpType.not_equal,
                    fill=0.0625, base=0,
                    pattern=[[-1, P]], channel_multiplier=1,
                )
                nc.sync.dma_start(dr8s[:], dr8_d[:])
                nc.sync.dma_start(dt8s[:], dt8_d[:])
            else:
                make_identity(nc, ident8[:])
            nc.vector.memset(vsb[:, :, :, HD], 1.0)

            for rep in range(reps):
                _emit_iteration(nc, tc, rep,
                                hst_d, wq_d, wk_d, wv_d, out_d,
                                drevt, dtt, maskt, bqkt,
                                identb, ident8, identf, qT, kT, vsb, ctxT,
                                outsb,
                                dr8s, dt8s, qT8f, kT8f, qT8, kT8,
                                hs8_d, wq8_d, wk8_d)

    nc.compile()
    _CACHE[key] = nc
    return nc


def _emit_iteration(nc, tc, rep, hst_d, wq_d, wk_d, wv_d, out_d,
                    drevt, dtt, maskt, bqkt,
                    identb, ident8, identf, qT, kT, vsb, ctxT, outsb,
                    dr8s, dt8s, qT8f, kT8f, qT8, kT8,
                    hs8_d, wq8_d, wk8_d):
    TAB = mybir.dt.float8e4 if OPTS["fp8_tab"] else BF16
    identt = ident8 if OPTS["fp8_tab"] else identb
    tab_bufs = OPTS.get("tab_bufs", 2) if OPTS["fp8_tab"] else 1
    # ---------------- Phase 1: projections ----------------
    with (
        tc.tile_pool(name=f"proj{rep}", bufs=1) as prp,
        tc.tile_pool(name=f"projps{rep}", bufs=OPTS.get("proj_bufs", 2), space="PSUM") as prps,
    ):
        hst = prp.tile([P, 6, S], BF16)
        wqt = prp.tile([P, 6, DW], BF16)
        wkt = prp.tile([P, 6, DW], BF16)
        wvt = prp.tile([P, 6, DW], BF16)
        pdr = OPTS["proj_dr"]
        if pdr:
            hs8t = prp.tile([P, 6, S], mybir.dt.float8e4)
            wq8t = prp.tile([P, 6, DW], mybir.dt.float8e4)
            wk8t = prp.tile([P, 6, DW], mybir.dt.float8e4)
        if OPTS.get("proj_order", False):
            # split loads so the first matmuls start early
            for kc in range(6):
                if pdr:
                    nc.sync.dma_start(hs8t[:, kc, :], hs8_d[:, kc, :])
                    nc.sync.dma_start(wq8t[:, kc, :], wq8_d[:, kc, :])
                    nc.sync.dma_start(wk8t[:, kc, :], wk8_d[:, kc, :])
                else:
                    nc.sync.dma_start(wqt[:, kc, :], wq_d[:, kc, :])
                    nc.sync.dma_start(wkt[:, kc, :], wk_d[:, kc, :])
                nc.sync.dma_start(hst[:, kc, :], hst_d[:, kc, :])
            nc.sync.dma_start(wvt[:], wv_d[:])
            # head-major: q then k per m-tile, so tables(0)/(1) unblock first
            order = [(pi, m) for m in range(3) for pi in range(2)]
        else:
            nc.sync.dma_start(hst[:], hst_d[:])
            nc.sync.dma_start(wqt[:], wq_d[:])
            nc.sync.dma_start(wkt[:], wk_d[:])
            nc.sync.dma_start(wvt[:], wv_d[:])
            order = [(pi, m) for pi in range(2) for m in range(3)]

        # qT / kT: out[M=128 outdims (2 heads), N=512 tokens]
        for pi, m in order:
            wt, dst = ((wqt, qT), (wkt, kT))[pi]
            w8 = (wq8t, wk8t)[pi] if pdr else None
            for nh2 in range(2):
                ps = prps.tile([P, 512], F32, tag="pqk")
                if pdr:
                    for g in range(3):
                        nc.tensor.matmul(
                            ps[:],
                            w8[:, 2 * g:2 * g + 2, m * P:(m + 1) * P],
                            hs8t[:, 2 * g:2 * g + 2,
                                 nh2 * 512:(nh2 + 1) * 512],
                            start=(g == 0), stop=(g == 2),
                            perf_mode=mybir.MatmulPerfMode.DoubleRow,
                            skip_group_check=True,
                        )
                else:
                    for kc in range(6):
                        nc.tensor.matmul(
                            ps[:],
                            wt[:, kc, m * P:(m + 1) * P],
                            hst[:, kc, nh2 * 512:(nh2 + 1) * 512],
                            start=(kc == 0), stop=(kc == 5),
                        )
                nc.scalar.activation(
                    dst[:, m, nh2 * 512:(nh2 + 1) * 512], ps[:],
                    AF.Identity, bias=bqkt[:, 3 * pi + m:3 * pi + m + 1],
                    scale=0.0625 if pdr else 1.0,
                )
                if OPTS["tab_dr"]:
                    f8 = (qT8f, kT8f)[pi]
                    nc.vector.tensor_scalar_add(
                        f8[:, m, nh2 * 512:(nh2 + 1) * 512], ps[:],
                        bqkt[:, 3 * pi + m:3 * pi + m + 1])
        # v: out[M=128 tokens, N=384 outdims]
        for t in range(NB):
            ps = prps.tile([P, DW], F32, tag="pv")
            for kc in range(6):
                nc.tensor.matmul(
                    ps[:],
                    hst[:, kc, t * P:(t + 1) * P],
                    wvt[:, kc, :],
                    start=(kc == 0), stop=(kc == 5),
                )
            nc.vector.tensor_copy(
                vsb[:, t, :, 0:HD],
                ps[:].rearrange("p (h d) -> p h d", h=HPC),
            )

    if OPTS["tab_dr"]:
        # regroup q/k fp8 into [32, 2(d-half), S] per head for DoubleRow:
        # head h lands on partitions (h%4)*32..+32, bank h//4
        for h in range(HPC):
            bb = h % 3
            for srcf, dstt in ((qT8f, qT8), (kT8f, kT8)):
                src = _flat_ap(
                    srcf[:], [[3 * S, 32], [32 * 3 * S, 2], [1, S]],
                    (h % 2) * 64 * 3 * S + (h // 2) * S)
                nc.scalar.dma_start(
                    dstt[bb * 32:bb * 32 + 32, h // 3, :, :], src)

    if OPTS["limit"] == "proj":
        return
    # ---------------- Phase 2: per-head attention ----------------
    shift_bufs = 1 + max(1, int(OPTS["lookahead"])) if OPTS["fp8_tab"] else 2
    with (
        tc.tile_pool(name=f"wtab{rep}", bufs=tab_bufs) as wtab,
        tc.tile_pool(name=f"wqpos{rep}", bufs=shift_bufs) as wqpos,
        tc.tile_pool(name=f"wkpt{rep}",
                     bufs=shift_bufs if OPTS["fp8_tab"] else 1) as wkpt,
        tc.tile_pool(name=f"wexp{rep}", bufs=OPTS.get("exp_bufs", 10)) as wexp,
        tc.tile_pool(name=f"hps{rep}", bufs=2, space="PSUM") as hps,
        tc.tile_pool(name=f"htab{rep}", bufs=1, space="PSUM") as htab,
        tc.tile_pool(name=f"cps{rep}", bufs=1, space="PSUM") as cps,
    ):
        CH = OPTS.get("tab_chunk", 384)   # table psum chunk: 384 f32 = 1 bank
        NCH = BAND // CH

        def emit_tables(h):
            base = (h % 2) * 64
            tl = h // 2
            qTh = qT[base:base + 64, tl, :]
            kTh = kT[base:base + 64, tl, :]
            # --- band tables + batched on-chip diagonal shifts
            atab = wtab.tile([P, NB, BAND], TAB, tag="atab")
            btab = wtab.tile([P, NB, BAND], TAB, tag="btab")
            pat = OPTS["evac_pat"]
            wb = OPTS.get("work_bufs", 4)
            for blk in range(NB):
                j0 = 896 - P * blk
                for side in range(2):
                    lhsT = (qTh if side == 0 else kTh)[:, blk * P:(blk + 1) * P]
                    rhs_t = (drevt if side == 0 else dtt)[base:base + 64, :]
                    tabs = (atab if side == 0 else btab)
                    bb = h % 3
                    lhsT8 = (qT8 if side == 0 else kT8)[
                        bb * 32:bb * 32 + 32, h // 3, :, blk * P:(blk + 1) * P]
                    d8 = (dr8s if side == 0 else dt8s)
                    for ci in range(NCH):
                        c0, c1 = ci * CH, (ci + 1) * CH
                        ps = hps.tile([P, CH], F32, tag="work", bufs=wb)
                        if OPTS["tab_dr"]:
                            for m0 in range(0, CH, 384):
                                m1 = min(m0 + 384, CH)
                                nc.tensor.matmul(
                                    ps[:, m0:m1], lhsT8,
                                    d8[bb * 32:bb * 32 + 32, :,
                                       j0 + c0 + m0:j0 + c0 + m1],
                                    start=True, stop=True,
                                    perf_mode=mybir.MatmulPerfMode.DoubleRow,
                                    skip_group_check=True,
                                )
                        else:
                            for m0 in range(0, CH, 512):
                                m1 = min(m0 + 512, CH)
                                nc.tensor.matmul(
                                    ps[:, m0:m1], lhsT,
                                    rhs_t[:, j0 + c0 + m0:j0 + c0 + m1],
                                    start=True, stop=True,
                                    skip_group_check=True,
                                )
                        eng = pat[(blk * 2 * NCH + side * NCH + ci) % len(pat)]
                        cp = (nc.scalar.copy if eng == "act" else
                              nc.gpsimd.tensor_copy if eng == "gp" else
                              nc.vector.tensor_copy)
                        cp(tabs[:, blk, c0:c1], ps[:])
            if OPTS["limit"] == "tabnoshift":
                nc.vector.tensor_copy(ctxT[0:P // 2, h, 0:S], atab[0:P // 2, 0, 0:S])
                nc.vector.tensor_copy(ctxT[0:1, h, 0:S], btab[0:1, 0, 0:S])
                return None
            # shift DMAs per side per head (optionally split by r-half so
            # the first score blocks unblock after half the transfer)
            qpos = wqpos.tile([P, NB, S], TAB, tag="qpos")
            kpt = wkpt.tile([P, NB, S], TAB, tag="kpt")
            nsp = int(OPTS.get("shift_split", 1))
            hw_ = S // nsp
            for sp in range(nsp):
                nc.sync.dma_start(
                    qpos[:, :, sp * hw_:(sp + 1) * hw_],
                    _diag_ap3(atab[:], NB, BAND, P, hw_, P - 1 + sp * hw_))
                nc.sync.dma_start(
                    kpt[:, :, sp * hw_:(sp + 1) * hw_],
                    _diag_ap3(btab[:], NB, BAND, P, hw_, P - 1 + sp * hw_))
            if OPTS["limit"] == "tables":
                nc.vector.tensor_copy(ctxT[0:P // 2, h, 0:S], qpos[0:P // 2, 0, :])
                nc.vector.tensor_copy(ctxT[0:1, h, 0:S], kpt[0:1, 0, :])
                return None
            return qpos, kpt

        def emit_scores(h, qpos, kpt):
            base = (h % 2) * 64
            tl = h // 2
            qTh = qT[base:base + 64, tl, :]
            kTh = kT[base:base + 64, tl, :]
            # --- scores in half-width (1-bank) psum tiles, softmax numerator
            cac = cps.tile([HD + 1, S], F32, tag="ctxacc")
            expts = []
            share = OPTS["tab_chunk"] == BAND
            for j in range(NB):
                expt = wexp.tile([P, S], BF16, tag="expt")
                if share:
                    sW = hps.tile([P, BAND], F32, tag="work", name="sW",
                                  bufs=OPTS.get("work_bufs", 2))
                    halves = [sW[:, nh2 * 512:(nh2 + 1) * 512]
                              for nh2 in range(2)]
                else:
                    halves = None
                for nh2 in range(2):
                    sS = halves[nh2] if share else hps.tile(
                        [P, 512], F32, tag="sc", name="sS",
                        bufs=OPTS.get("sc_bufs", 2))
                    nc.tensor.matmul(
                        sS[:],
                        kTh[:, j * P:(j + 1) * P],
                        qTh[:, nh2 * 512:(nh2 + 1) * 512],
                        start=True, stop=False,
                        skip_group_check=True,
                    )
                    for bi in range(4):
                        blk = nh2 * 4 + bi
                        nc.tensor.matmul(
                            sS[:, bi * P:(bi + 1) * P],
                            qpos[:, blk, j * P:(j + 1) * P],
                            identt[:],
                            start=False, stop=False,
                            skip_group_check=True,
                        )
                    nc.tensor.matmul(
                        sS[:],
                        identt[:],
                        kpt[:, j, nh2 * 512:(nh2 + 1) * 512],
                        start=False, stop=True,
                        skip_group_check=True,
                    )
                    if not share:
                        nc.scalar.activation(
                            expt[:, nh2 * 512:(nh2 + 1) * 512], sS[:], AF.Exp,
                            bias=maskt[:, j:j + 1], scale=0.125)
                if share:
                    nc.scalar.activation(
                        expt[:], sW[:, 0:S], AF.Exp,
                        bias=maskt[:, j:j + 1], scale=0.125)
                expts.append(expt)

            # --- PV (+denominator via ones column)
            if OPTS["limit"] == "scores":
                return
            for j in range(NB):
                for nh2 in range(2):
                    nc.tensor.matmul(
                        cac[:, nh2 * 512:(nh2 + 1) * 512],
                        vsb[:, j, h, :],
                        expts[j][:, nh2 * 512:(nh2 + 1) * 512],
                        start=(j == 0), stop=(j == NB - 1),
                        skip_group_check=True,
                    )
            if OPTS["ctx_gp"]:
                nc.gpsimd.tensor_copy(ctxT[:, h, :], cac[:])
            elif OPTS["ctx_dve"]:
                nc.vector.tensor_copy(ctxT[:, h, :], cac[:])
            else:
                nc.scalar.copy(ctxT[:, h, :], cac[:])

        depth = int(OPTS["lookahead"])
        if depth > 0:
            pend = {}
            for h in range(min(depth, HPC)):
                pend[h] = emit_tables(h)
            for h in range(HPC):
                if h + depth < HPC:
                    pend[h + depth] = emit_tables(h + depth)
                sh = pend.pop(h)
                if sh is not None:
                    emit_scores(h, *sh)
        else:
            for h in range(HPC):
                sh = emit_tables(h)
                if sh is not None:
                    emit_scores(h, *sh)

        if OPTS["fin_inline"] and OPTS["limit"] == "full":
            nc.sync.dma_start(out_d[:], outsb[:])

    if OPTS["limit"] != "full" or OPTS["fin_inline"]:
        return
    # ---------------- Phase 3: output assembly ----------------
    with (
        tc.tile_pool(name=f"fin{rep}", bufs=4) as fin,
        tc.tile_pool(name=f"fps{rep}", bufs=4, space="PSUM") as fps,
    ):
        for h in range(HPC):
            for lt in range(NB):
                if OPTS["fin_bf16"]:
                    ct = fps.tile([P, HD + 1], BF16, tag="ctps")
                    nc.tensor.transpose(
                        ct[:],
                        ctxT[:, h, lt * P:(lt + 1) * P],
                        identb[0:HD + 1, 0:HD + 1],
                    )
                else:
                    ct = fps.tile([P, HD + 1], F32, tag="ctps")
                    nc.tensor.matmul(
                        ct[:],
                        ctxT[:, h, lt * P:(lt + 1) * P],
                        identf[0:HD + 1, 0:HD + 1],
                        start=True, stop=True,
                        skip_group_check=True,
                    )
                rc = fin.tile([P, 1], F32, tag="rc")
                nc.vector.reciprocal(rc[:], ct[:, HD:HD + 1])
                nc.vector.tensor_scalar_mul(
                    outsb[:, lt, h * HD:(h + 1) * HD],
                    ct[:, 0:HD], rc[:])
        nc.sync.dma_start(out_d[:], outsb[:])


def build_in_maps(inputs):
    hs = np.asarray(inputs["hidden_states"], np.float32)
    am = np.asarray(inputs["attention_mask"], np.float32)
    Wq = np.asarray(inputs["Wq"], np.float32)
    Wk = np.asarray(inputs["Wk"], np.float32)
    Wv = np.asarray(inputs["Wv"], np.float32)
    bq = np.asarray(inputs["bq"], np.float32)
    bk = np.asarray(inputs["bk"], np.float32)
    de = np.asarray(inputs["dist_emb"], np.float32)

    bf = ml_dtypes.bfloat16

    # dist tables, padded to 2048 cols, duplicated on both partition halves
    drevt = np.zeros((64, JW), np.float32)
    drevt[:, :2047] = de[::-1].T
    dtt = np.zeros((64, JW), np.float32)
    dtt[:, :2047] = de.T
    drevt = np.concatenate([drevt, drevt], 0).astype(bf)
    dtt = np.concatenate([dtt, dtt], 0).astype(bf)

    # DoubleRow fp8 D tables, 16x-scaled (1/16 folded into inject identity):
    # [bank*32+ki, ko, j] = 16*D'[j, ko*32+ki], replicated over 4 banks
    fp8 = ml_dtypes.float8_e4m3fn
    xr = np.zeros((64, JW), np.float32)
    xr[:, :2047] = 16.0 * de[::-1].T
    dr8 = np.tile(xr.reshape(2, 32, JW).transpose(1, 0, 2), (4, 1, 1)).astype(fp8)
    xt = np.zeros((64, JW), np.float32)
    xt[:, :2047] = 16.0 * de.T
    dt8 = np.tile(xt.reshape(2, 32, JW).transpose(1, 0, 2), (4, 1, 1)).astype(fp8)

    in_maps = []
    for core in range(NCORES):
        b, g = divmod(core, 2)
        cols = slice(g * DW, (g + 1) * DW)
        hst = np.ascontiguousarray(hs[b].T).reshape(6, P, S)
        hst = np.ascontiguousarray(hst.transpose(1, 0, 2)).astype(bf)
        wqc = np.ascontiguousarray(
            Wq[:, cols].reshape(6, P, DW).transpose(1, 0, 2)).astype(bf)
        wkc = np.ascontiguousarray(
            Wk[:, cols].reshape(6, P, DW).transpose(1, 0, 2)).astype(bf)
        wvc = np.ascontiguousarray(
            Wv[:, cols].reshape(6, P, DW).transpose(1, 0, 2)).astype(bf)
        maskc = np.ascontiguousarray(am[b, 0, 0, :].reshape(NB, P).T)
        # col 0..2: bq m-tiles (128 dims each); col 3..5: bk m-tiles
        bqkc = np.concatenate(
            [bq[cols].reshape(3, P).T, bk[cols].reshape(3, P).T], axis=1)
        hs8 = hst.astype(np.float32).astype(fp8)
        wq8c = (16.0 * wqc.astype(np.float32)).astype(fp8)
        wk8c = (16.0 * wkc.astype(np.float32)).astype(fp8)
        in_maps.append({
            "hst": hst, "wq": wqc, "wk": wkc, "wv": wvc,
            "hs8": hs8, "wq8": wq8c, "wk8": wk8c,
            "drevt": drevt, "dtt": dtt, "dr8": dr8, "dt8": dt8,
            "maskc": maskc.astype(np.float32),
            "bqkc": np.ascontiguousarray(bqkc).astype(np.float32),
        })
    return in_maps


def kernel(hidden_states, attention_mask, Wq, bq, Wk, bk, Wv, bv, dist_emb):
    in_maps = build_in_maps({
        "hidden_states": hidden_states, "attention_mask": attention_mask,
        "Wq": Wq, "Wk": Wk, "Wv": Wv, "bq": bq, "bk": bk,
        "dist_emb": dist_emb,
    })
    bv = np.asarray(bv, np.float32)

    nc = _build()
    try:
        res = run_bass_kernel_spmd(nc, in_maps, core_ids=list(range(NCORES)))
    except Exception:
        # one retry for transient runtime/device hiccups
        res = run_bass_kernel_spmd(nc, in_maps, core_ids=list(range(NCORES)))

    out = np.empty((B, S, H), np.float32)
    for core in range(NCORES):
        b, g = divmod(core, 2)
        o = res.results[core]["out"]          # [128, 8, 384]
        out[b, :, g * DW:(g + 1) * DW] = o.transpose(1, 0, 2).reshape(S, DW)
    out += bv[None, None, :]
    return out

